# revision 4
# baseline (speedup 1.0000x reference)
"""BiLSTM-CRF Trainium2 kernel, v2 (direction-split, transposed gates).

Sharding: 8 cores = 4 sentence-groups x 2 directions. Core (d, g) runs
direction d (0=fwd, 1=bwd) of sentences [16g, 16g+16) and produces the
partial CRF tensor for its direction: fwd partial includes transition +
b_lin; bwd partial is just the bwd emission broadcast over i. kernel()
sums fwd+bwd partials per group (the unshard step) after flipping the
bwd core's time axis (bwd cores run in reversed "virtual time" v so the
program is SPMD-uniform).

Per-core device pipeline:
  1. Embedding gather (indirect DMA) in v-order, PE-transpose into
     xT [128=E, L*32] bf16 (32 cols per step: 16 real sentences + 16
     zero pads so per-step matmul output rows land on 32-partition
     boundaries).
  2. Scan, 4 steps per PSUM window [128, 1024]: window opens with
     x-side matmuls (xT_win.T @ WihT) + bias (ones.T @ bias row), then
     per step v: 4 accumulating matmuls add WhhT.T-side contribution
     for rows [32*(v%4), +32); sigmoid ACT over [32, 0:768] (i,f,o),
     tanh ACT over [32, 768:1024] (g), DVE cell update, h cast to bf16,
     2 transpose-DMAs store hT into h_all [128, (L+1)*64].
  3. Emission: per 128-token tile (8 v x 16 real b), 2 accumulating
     matmuls vs WlinT chunks -> eps [128, 32]; CRF = DVE broadcast add
     (eps repeated over i via stride-0 AP) + trans_rep; DMA out.
"""

import numpy as np

VOCAB, EMB, HID, OUT = 30000, 128, 256, 32
B, L = 64, 512
NCORES = 8
BC = 16           # sentences per core
BP = 32           # padded batch per step (16 real + 16 pad)
WIN = 4           # steps per PSUM window
USE_DMAT = True   # dma_start_transpose for hT
PADE_TANH = False  # DVE has no tensor divide; ACT tanh stays
FP8H = False      # no speedup in this runtime (flat per-instr cost), 5x worse error


def _host_prep(inputs, L_eff=L):
    import ml_dtypes

    sents = np.asarray(inputs["sents_tensor"]).astype(np.int32)  # [B, L]
    emb = np.asarray(inputs["embedding"]).astype(np.float32)

    # torch gate order i,f,g,o (256 each) -> ours i,f,o,g
    perm = np.concatenate([np.arange(0, 512), np.arange(768, 1024),
                           np.arange(512, 768)])

    def dir_consts(Wih, Whh, bih, bhh, Wlin_half, trans_add):
        Wih = np.asarray(Wih, np.float32)[perm].copy()   # [1024, 128]
        Whh = np.asarray(Whh, np.float32)[perm].copy()   # [1024, 256]
        bias = (np.asarray(bih, np.float32)
                + np.asarray(bhh, np.float32))[perm].copy()
        # g block scaled by 2: tanh(g) computed as 2*sigmoid(2g)-1 so one
        # sigmoid ACT covers all four gates
        Wih[768:1024] *= 2.0
        Whh[768:1024] *= 2.0
        bias[768:1024] *= 2.0
        WhhT = np.ascontiguousarray(Whh.T)           # [256, 1024]
        WihT = np.ascontiguousarray(Wih.T)           # [128, 1024]
        WlinT = np.ascontiguousarray(Wlin_half.T)    # [256, 32]
        c2 = np.zeros([128, 4160], np.float32)
        for kt in range(2):
            c2[:, kt * 1024:(kt + 1) * 1024] = WhhT[kt * 128:(kt + 1) * 128]
        c2[:, 2048:3072] = WihT
        for kt in range(2):
            c2[:, 3072 + kt * 32:3072 + (kt + 1) * 32] = \
                WlinT[kt * 128:(kt + 1) * 128]
        c2[0, 3136:4160] = bias
        c8 = np.zeros([128, 2048], np.float32)
        for ko in range(2):
            c8[:, ko * 1024:(ko + 1) * 1024] = WhhT[ko * 128:(ko + 1) * 128]
        return (c2.astype(ml_dtypes.bfloat16),
                c8.astype(ml_dtypes.float8_e4m3),
                trans_add.astype(np.float32))

    W_lin = np.asarray(inputs["W_lin"], np.float32)      # [32, 512]
    b_lin = np.asarray(inputs["b_lin"], np.float32)
    trans = np.asarray(inputs["transition"], np.float32)

    trans_f = np.broadcast_to(
        (trans + b_lin[None, :]).reshape(1, -1), (128, 1024)).copy()
    trans_b = np.zeros([128, 1024], np.float32)

    c2_f, c8_f, tr_f = dir_consts(inputs["Wih_f"], inputs["Whh_f"],
                                  inputs["bih_f"], inputs["bhh_f"],
                                  W_lin[:, 0:256], trans_f)
    c2_b, c8_b, tr_b = dir_consts(inputs["Wih_b"], inputs["Whh_b"],
                                  inputs["bih_b"], inputs["bhh_b"],
                                  W_lin[:, 256:512], trans_b)

    NG = L_eff * BC // 128  # gather tiles
    in_maps = []
    for core in range(NCORES):
        d, g = divmod(core, 4)
        S = sents[16 * g:16 * g + 16, :L_eff]   # [16, L]
        # v-order tokens: tok(v, b) = S[b, v] fwd / S[b, L-1-v] bwd
        Sv = S.T if d == 0 else S.T[::-1]       # [L, 16], row v
        idx = np.ascontiguousarray(
            Sv.reshape(NG, 128).T).astype(np.int32)  # [128, NG] p=8v'+... p = v_l*16+b
        c4 = np.zeros([128, NG + 1024], np.uint32)
        c4[:, 0:NG] = idx.view(np.uint32)
        c4[:, NG:NG + 1024] = (tr_f if d == 0 else tr_b).view(np.uint32)
        m = {
            "c4": np.ascontiguousarray(c4),
            "c2": np.ascontiguousarray(c2_f if d == 0 else c2_b),
            "emb": emb,
        }
        if FP8H:
            m["c8"] = np.ascontiguousarray(c8_f if d == 0 else c8_b)
        in_maps.append(m)
    return in_maps


def build_nc(L_eff=L, reps=1, timing=False, with_bias=False):
    import concourse.bass as bass
    import concourse.mybir as mybir
    import concourse.tile as tile
    from concourse.bacc import Bacc
    from concourse.masks import make_identity

    dt = mybir.dt
    AF = mybir.ActivationFunctionType
    OP = mybir.AluOpType

    NG = L_eff * BC // 128          # gather tiles
    NW = L_eff // WIN               # PSUM windows
    NE = L_eff // 4                 # emission tiles (4 v x 32 padded b)

    nc = Bacc()
    d_c4 = nc.declare_dram_parameter("c4", [128, NG + 1024], dt.uint32, False)
    d_c2 = nc.declare_dram_parameter("c2", [128, 4160], dt.bfloat16, False)
    if FP8H:
        d_c8 = nc.declare_dram_parameter("c8", [128, 2048], dt.float8e4,
                                         False)
    if timing:
        d_emb = nc.dram_tensor("embt", [VOCAB, EMB], dt.float32)
        d_out = nc.dram_tensor("outt", [L_eff // 4, 128, 1024], dt.float32)
        d_out_ext = nc.declare_dram_parameter("out", [1, 16], dt.float32,
                                              isOutput=True)
    else:
        d_emb = nc.declare_dram_parameter("emb", [VOCAB, EMB], dt.float32,
                                          False)
        d_out = nc.declare_dram_parameter("out", [L_eff // 4, 128, 1024],
                                          dt.float32, isOutput=True)
        d_out_ext = None

    with tile.TileContext(nc) as tc:
        with (
            tc.tile_pool(name="const", bufs=1) as const,
            tc.tile_pool(name="state", bufs=1) as state,
            tc.tile_pool(name="gat", bufs=3) as gat,
            tc.tile_pool(name="misc_ps", bufs=2, space="PSUM") as misc_ps,
            tc.tile_pool(name="gw_ps", bufs=1 if FP8H else 2,
                         space="PSUM") as gw_ps,
            tc.tile_pool(name="gst_ps", bufs=2, space="PSUM") as gst_ps,
            tc.tile_pool(name="work", bufs=2) as work,
            tc.tile_pool(name="crf_sb", bufs=3) as crf_sb_p,
        ):
            ident = const.tile([128, 128], dt.float32)
            make_identity(nc, ident[:])
            ident_bf = const.tile([128, 128], dt.bfloat16)
            nc.vector.tensor_copy(out=ident_bf[:], in_=ident[:])
            ones1 = const.tile([1, 128], dt.bfloat16)
            nc.vector.memset(ones1[:], 1.0)
            c4_sb = const.tile([128, NG + 1024], dt.uint32)
            nc.sync.dma_start(out=c4_sb[:], in_=d_c4[:])
            idx_sb = c4_sb[:, 0:NG].bitcast(dt.int32)
            trans_rep = c4_sb[:, NG:NG + 1024].bitcast(dt.float32)
            c2_sb = const.tile([128, 4160], dt.bfloat16)
            nc.sync.dma_start(out=c2_sb[:], in_=d_c2[:])
            if FP8H:
                c8_sb = const.tile([128, 2048], dt.float8e4)
                nc.sync.dma_start(out=c8_sb[:], in_=d_c8[:])

                def whh8(hf):  # [128, 2, 512] DoubleRow moving operand
                    return c8_sb[:].rearrange("p (k n) -> p k n", k=2)[
                        :, :, hf * 512:(hf + 1) * 512]

            def whhT(kt, hf):  # [128, 512] moving operand
                return c2_sb[:, kt * 1024 + hf * 512:kt * 1024 + (hf + 1) * 512]

            def wihT(hf):
                return c2_sb[:, 2048 + hf * 512:2048 + (hf + 1) * 512]

            def wlinT(kt):
                return c2_sb[:, 3072 + kt * 32:3072 + (kt + 1) * 32]

            bias_row = c2_sb[0:1, 3136:4160]  # [1, 1024]

            # persistent buffers
            xT = state.tile([128, L_eff * BP], dt.bfloat16)
            # kt-major so emission reads one contiguous free dim per chunk
            h_all = state.tile([128, 2 * (L_eff + 1) * BP], dt.bfloat16)
            c_st = state.tile([BP, 256], dt.float32)
            nc.vector.memset(h_all[:, 0:BP], 0.0)
            nc.vector.memset(
                h_all[:, (L_eff + 1) * BP:(L_eff + 2) * BP], 0.0)

            def hslot(v, kt):  # stationary hT [128, 32] for step v
                off = (kt * (L_eff + 1) + v) * BP
                return h_all[:, off:off + BP]

            # ---- phase 1: gather + transpose into xT (outside reps) ----
            nc.vector.memset(xT[:], 0.0)
            for gi in range(0 if timing else NG):
                gt = gat.tile([128, 128], dt.float32, tag="g")
                nc.gpsimd.indirect_dma_start(
                    out=gt[:], out_offset=None, in_=d_emb[:],
                    in_offset=bass.IndirectOffsetOnAxis(
                        ap=idx_sb[:, gi:gi + 1], axis=0))
                tp = misc_ps.tile([128, 128], dt.float32, tag="m")
                nc.tensor.transpose(out=tp[:], in_=gt[:], identity=ident[:])
                # dst: cols v0*BP + v_l*BP + b  (v_l in 0..8, b in 0..16)
                dst = xT[:, gi * 8 * BP:(gi + 1) * 8 * BP].rearrange(
                    "p (v c) -> p v c", v=8)[:, :, 0:16]
                nc.vector.tensor_copy(
                    out=dst,
                    in_=tp.rearrange("p (v c) -> p v c", v=8))

            def scan_and_emit():
                for w in range(NW):
                    gps = gw_ps.tile([128, 1024], dt.float32, tag="gw")
                    for hf in range(2):
                        nc.tensor.matmul(
                            out=gps[:, hf * 512:(hf + 1) * 512],
                            lhsT=xT[:, w * 128:(w + 1) * 128],
                            rhs=wihT(hf), start=True,
                            stop=FP8H and not with_bias,
                            skip_group_check=True)
                        if with_bias:
                            nc.tensor.matmul(
                                out=gps[:, hf * 512:(hf + 1) * 512],
                                lhsT=ones1[:], rhs=bias_row[:, hf * 512:
                                                            (hf + 1) * 512],
                                start=False, stop=False,
                                skip_group_check=True)
                    if FP8H:
                        P_sb = work.tile([128, 1024], dt.bfloat16, tag="P")
                        nc.vector.tensor_copy(out=P_sb[:], in_=gps[:])
                    for s in range(WIN):
                        v = w * WIN + s
                        r0 = s * BP
                        if FP8H:
                            hf8 = work.tile([128, 64], dt.float8e4,
                                            tag="hf8")
                            nc.vector.tensor_copy(
                                out=hf8[:],
                                in_=h_all[:].rearrange(
                                    "p (k s c) -> p k s c",
                                    k=2, s=L_eff + 1)[:, :, v, :])
                            g_st = gst_ps.tile([BP, 1024], dt.float32,
                                               tag="gst")
                            mybir_dr = mybir.MatmulPerfMode.DoubleRow
                            for hf in range(2):
                                nc.tensor.matmul(
                                    out=g_st[:, hf * 512:(hf + 1) * 512],
                                    lhsT=hf8[:].rearrange(
                                        "p (k m) -> p k m", k=2),
                                    rhs=whh8(hf),
                                    perf_mode=mybir_dr,
                                    start=True, stop=True,
                                    skip_group_check=True)
                            gsb = work.tile([BP, 1024], dt.float32,
                                            tag="gsb")
                            nc.vector.tensor_tensor(
                                out=gsb[:], in0=g_st[:],
                                in1=P_sb[r0:r0 + BP, :], op=OP.add)
                        else:
                            for hf in range(2):
                                for kt in range(2):
                                    nc.tensor.matmul(
                                        out=gps[r0:r0 + BP,
                                                hf * 512:(hf + 1) * 512],
                                        lhsT=hslot(v, kt), rhs=whhT(kt, hf),
                                        start=False,
                                        stop=(s == WIN - 1 and hf == 1
                                              and kt == 1),
                                        skip_group_check=True,
                                        tile_position=(0, r0))
                        sig = work.tile([BP, 1024], dt.float32, tag="sig")
                        nc.scalar.activation(
                            out=sig[:],
                            in_=gsb[:] if FP8H else gps[r0:r0 + BP, 0:1024],
                            func=AF.Sigmoid)
                        thg = work.tile([BP, 256], dt.float32, tag="thg")
                        nc.vector.tensor_scalar(out=thg[:],
                                                in0=sig[:, 768:1024],
                                                scalar1=2.0, scalar2=-1.0,
                                                op0=OP.mult, op1=OP.add)
                        a = work.tile([BP, 256], dt.float32, tag="a")
                        nc.vector.tensor_tensor(out=a[:], in0=sig[:, 0:256],
                                                in1=thg[:], op=OP.mult)
                        if v > 0:
                            bb = work.tile([BP, 256], dt.float32, tag="b")
                            nc.vector.tensor_tensor(out=bb[:],
                                                    in0=sig[:, 256:512],
                                                    in1=c_st[:], op=OP.mult)
                            nc.vector.tensor_tensor(out=c_st[:], in0=a[:],
                                                    in1=bb[:], op=OP.add)
                        else:
                            nc.vector.tensor_copy(out=c_st[:], in_=a[:])
                        th = work.tile([BP, 256], dt.float32, tag="th")
                        if PADE_TANH:
                            # tanh(x) ~ x(27+x^2)/(27+9x^2), |err|<2e-3 for
                            # |x|<1.5 (c stays well inside)
                            x2 = work.tile([BP, 256], dt.float32, tag="x2")
                            nc.vector.tensor_tensor(out=x2[:], in0=c_st[:],
                                                    in1=c_st[:], op=OP.mult)
                            num = work.tile([BP, 256], dt.float32, tag="num")
                            nc.vector.scalar_tensor_tensor(
                                out=num[:], in0=x2[:], scalar=27.0,
                                in1=c_st[:], op0=OP.add, op1=OP.mult)
                            den = work.tile([BP, 256], dt.float32, tag="den")
                            nc.vector.tensor_scalar(
                                out=den[:], in0=x2[:], scalar1=9.0,
                                scalar2=27.0, op0=OP.mult, op1=OP.add)
                            nc.vector.tensor_tensor(out=th[:], in0=num[:],
                                                    in1=den[:],
                                                    op=OP.divide)
                        else:
                            nc.scalar.activation(out=th[:], in_=c_st[:],
                                                 func=AF.Tanh)
                        h = work.tile([BP, 256],
                                      dt.bfloat16 if USE_DMAT else dt.float32,
                                      tag="h")
                        nc.vector.tensor_tensor(out=h[:], in0=sig[:, 512:768],
                                                in1=th[:], op=OP.mult)
                        if USE_DMAT:
                            # one transpose DMA fans out to both kt-major
                            # slots: out rows (k, p) map to h cols k*128+p
                            nc.sync.dma_start_transpose(
                                out=h_all[:].rearrange(
                                    "p (k s c) -> p k s c",
                                    k=2, s=L_eff + 1)[:, :, v + 1, :],
                                in_=h[:])
                        for kt in range(2):
                            if USE_DMAT:
                                break
                                tps = tr_ps.tile([128, 32], dt.float32,
                                                 tag="tr")
                                nc.tensor.transpose(
                                    out=tps[:], in_=h[:, kt * 128:
                                                      (kt + 1) * 128],
                                    identity=ident[0:BP, 0:BP])
                                nc.vector.tensor_copy(out=hslot(v + 1, kt),
                                                      in_=tps[:])

                # ---- emission + CRF (4 v per tile, pad rows included) ----
                for e in range(NE):
                    v0 = e * 4
                    eps = misc_ps.tile([128, 32], dt.float32, tag="m")
                    for kt in range(2):
                        off = (kt * (L_eff + 1) + v0 + 1) * BP
                        nc.tensor.matmul(out=eps[:],
                                         lhsT=h_all[:, off:off + 4 * BP],
                                         rhs=wlinT(kt), start=(kt == 0),
                                         stop=(kt == 1))
                    crf = crf_sb_p.tile([128, 1024], dt.float32, tag="c")
                    eps_b = eps[:].rearrange("p (o j) -> p o j",
                                             o=1).broadcast_to((128, 32, 32))
                    nc.vector.tensor_tensor(
                        out=crf[:], in0=eps_b,
                        in1=trans_rep.rearrange("p (i j) -> p i j", i=32),
                        op=OP.add)
                    # full padded tile out; host strips pad rows
                    nc.sync.dma_start(out=d_out[e], in_=crf[:])

            for _rep in range(reps):
                scan_and_emit()

            if timing:
                tl = crf_sb_p.tile([1, 16], dt.float32, tag="tl")
                nc.sync.dma_start(out=tl[:], in_=d_out[0, 0, 0:16])
                nc.sync.dma_start(out=d_out_ext[:], in_=tl[:])

    nc.finalize()
    return nc


_CACHE = {}


def _get_nc(L_eff=L, with_bias=False):
    key = (L_eff, with_bias)
    if key not in _CACHE:
        _CACHE[key] = build_nc(L_eff, with_bias=with_bias)
    return _CACHE[key]


def kernel(**inputs):
    from concourse.bass_utils import run_bass_kernel_spmd

    L_eff = np.asarray(inputs["sents_tensor"]).shape[1]
    with_bias = any(
        np.any(np.asarray(inputs[k])) for k in
        ("bih_f", "bhh_f", "bih_b", "bhh_b"))
    nc = _get_nc(L_eff, with_bias)
    in_maps = _host_prep(inputs, L_eff)
    res = run_bass_kernel_spmd(nc, in_maps, list(range(NCORES)))

    def unpack(o):
        # o: [L/4, 128, 1024]; rows = (v_l 4, b 32) with b<16 real
        o = o.reshape(L_eff // 4, 4, 32, 1024)[:, :, 0:16, :]
        return o.reshape(L_eff, BC, 1024).transpose(1, 0, 2)  # [BC, L, 1024]

    out = np.zeros([B, L_eff, OUT, OUT], np.float32)
    for g in range(4):
        f = unpack(res.results[g]["out"])
        bwd = unpack(res.results[4 + g]["out"])[:, ::-1]
        out[16 * g:16 * g + 16] = (f + bwd).reshape(BC, L_eff, OUT, OUT)
    return out


if __name__ == "__main__":
    nc = build_nc(64)
    print("built OK")
